# revision 4
# baseline (speedup 1.0000x reference)
"""Trainium2 Bass kernel for the DeformableSDFHead MLP.

Network (per point n, 16 bones k):
  x_k = [xyz3_k (3) | gl (48)]            gl shared per group of 4096 points
  h0  = relu(W0_k x_k + b0_k)             51 -> 64
  h_{l+1} = relu(Wmid_{k,l} h_l + bmid + h_l)   l = 0..6  (residual)
  latent = pre-residual out of l=6
  y = tanh(concat_k(latent_k) . Wf + bf)

Restructuring (all folds done host-side in numpy):
  * gl part of layer 0 folded into a per-(bone, group) bias beff.
  * residual folded into weights: W' = Wmid + I for l=0..5.
  * layer l=6 + final projection folded into a per-bone matvec:
      y = tanh(sum_k v_k . h6_k + c),  v_k = Wmid_{k,6}^T Wf_k.

Mapping: data-parallel over 8 cores (8192 points each). Bones are packed 2
per 128-partition tile (8 pairs); mid-layer matmuls use the 2x2 quadrant
structure of the PE array (crossed placement for odd pairs, absorbed into
the _SIGMA weight permutation). The kernel is PSUM-evacuation-bound: every
activation crosses PSUM fp32 -> SBUF fp16 through ScalarE (ACT) or VectorE
(DVE) at 1 elem/cycle/lane, so the build loop statically load-balances
half-pair [128,1024] relu+bias jobs across the two engines by simulated
engine time, with each engine ping-ponging two private 2-bank PSUM slots.
The final matvec borrows a DVE-slot generation between supergroups.
"""

import numpy as np

import concourse.bacc as bacc
import concourse.bass as bass
import concourse.mybir as mybir
from concourse.tile import TileContext
from concourse.bass_utils import run_bass_kernel_spmd

NUM_BONES = 16
HID = 64
JOINT_IDX = np.array([0, 1, 2, 3, 5, 6, 7, 9, 10, 11, 13, 14, 15, 17, 18, 19])

NCORES = 8
N = 65536
NS = N // NCORES       # 8192 points per core
SG = 2048              # supergroup (points held in SBUF per pipeline stage)
NSG = NS // SG         # 4
F = 512                # matmul free-dim chunk (one psum bank)
JD = 1024              # evacuation job free dim (half pair, 2 psum banks)

FP16 = mybir.dt.float16
FP32 = mybir.dt.float32

# measured per-op costs (ns) used for static ACT/DVE load balancing
ACT_RELU_1024 = 1114
DVE_RELU_1024 = 1283
MV_COPY_512 = 700

_SIGMA = [[(2 * p, 2 * p + 1) for p in range(8)]]
for _l in range(6):
    _SIGMA.append([_SIGMA[-1][p] if p % 2 == 0 else _SIGMA[-1][p][::-1]
                   for p in range(8)])


def _host_prep(xyz, joints, W0, b0, Wmid, bmid, Wf, bf):
    f32 = np.float32
    B = joints.shape[0]
    gl = joints[:, JOINT_IDX, :]
    gl = (gl - gl[:, :1, :]).reshape(B, -1).astype(f32)

    W0a = W0[:, :, 0:3].astype(f32)
    W0b = W0[:, :, 3:].astype(f32)
    beff = b0[:, None, :].astype(f32) + np.einsum('gi,koi->kgo', gl, W0b)

    I = np.eye(HID, dtype=f32)
    Wm_f = Wmid[:, :6].astype(f32) + I[None, None]

    Wf_k = Wf.reshape(NUM_BONES, HID).astype(f32)
    v = np.einsum('koi,ko->ki', Wmid[:, 6].astype(f32), Wf_k)
    c = float(np.sum(bmid[:, 6].astype(f32) * Wf_k) + bf[0])

    wm = np.zeros((128, 6 * 8 * 64), dtype=f32)
    bm = np.zeros((128, 48), dtype=f32)
    for l in range(6):
        for p in range(8):
            col = (l * 8 + p) * 64
            blo, bhi = _SIGMA[l][p]
            wm[0:64, col:col + 64] = Wm_f[blo, l].T
            wm[64:128, col:col + 64] = Wm_f[bhi, l].T
            olo, ohi = _SIGMA[l + 1][p]
            bm[0:64, l * 8 + p] = bmid[olo, l]
            bm[64:128, l * 8 + p] = bmid[ohi, l]

    w0 = np.zeros((128, 256), dtype=f32)
    for g in range(4):
        for j in range(4):
            w0[32 * j:32 * j + 3, 64 * g:64 * (g + 1)] = W0a[4 * g + j].T

    vt = np.zeros((128, 8 * 32), dtype=f32)
    for p in range(8):
        blo, bhi = _SIGMA[6][p]
        vt[0:64, 32 * p:32 * p + 32] = v[blo][:, None]
        vt[64:128, 32 * p:32 * p + 32] = v[bhi][:, None]

    xyzf = xyz.astype(f32)
    in_maps = []
    for core in range(NCORES):
        n0 = core * NS
        x3 = np.zeros((4, 12, NS), dtype=np.float16)
        for g in range(4):
            for j in range(4):
                b_ = 4 * g + j
                x3[g, 3 * j:3 * j + 3, :] = (
                    xyzf[n0:n0 + NS, 3 * (b_ + 1):3 * (b_ + 1) + 3].T.astype(np.float16))
        b0e = np.zeros((128, 16), dtype=f32)
        for p in range(8):
            blo, bhi = _SIGMA[0][p]
            for gi in range(2):
                grp = 2 * core + gi
                b0e[0:64, p * 2 + gi] = beff[blo, grp]
                b0e[64:128, p * 2 + gi] = beff[bhi, grp]
        in_maps.append(dict(
            x3=x3,
            w0=w0.astype(np.float16),
            wm=wm.astype(np.float16),
            bm=bm,
            b0e=b0e,
            vt=vt.astype(np.float16),
        ))
    return in_maps, c


_CACHE = {}


def _build():
    nc = bacc.Bacc("TRN2", target_bir_lowering=False)

    x3_h = nc.dram_tensor("x3", [4, 12, NS], FP16, kind="ExternalInput")
    w0_h = nc.dram_tensor("w0", [128, 256], FP16, kind="ExternalInput")
    wm_h = nc.dram_tensor("wm", [128, 6 * 8 * 64], FP16, kind="ExternalInput")
    bm_h = nc.dram_tensor("bm", [128, 48], FP32, kind="ExternalInput")
    b0e_h = nc.dram_tensor("b0e", [128, 16], FP32, kind="ExternalInput")
    vt_h = nc.dram_tensor("vt", [128, 8 * 32], FP16, kind="ExternalInput")
    out_h = nc.dram_tensor("out", [NSG, 4, F], FP32, kind="ExternalOutput")

    Relu = mybir.ActivationFunctionType.Relu
    ADD = mybir.AluOpType.add
    MAX = mybir.AluOpType.max

    with TileContext(nc) as tc:
        with (
            tc.tile_pool(name="const", bufs=1) as cpool,
            tc.tile_pool(name="xin", bufs=2) as xpool,
            tc.tile_pool(name="hbuf", bufs=2) as hpool,
            tc.tile_pool(name="outp", bufs=2) as opool,
            tc.tile_pool(name="psa", bufs=2, space="PSUM") as papool,
            tc.tile_pool(name="psv", bufs=2, space="PSUM") as pvpool,
        ):
            w0_t = cpool.tile([128, 256], FP16, name="w0t")
            wm_t = cpool.tile([128, 6 * 8 * 64], FP16, name="wmt")
            bm_t = cpool.tile([128, 48], FP32, name="bmt")
            b0e_t = cpool.tile([128, 16], FP32, name="b0et")
            vt_t = cpool.tile([128, 8 * 32], FP16, name="vtt")
            nc.sync.dma_start(out=w0_t[:, :], in_=w0_h[:, :])
            nc.sync.dma_start(out=b0e_t[:, :], in_=b0e_h[:, :])

            # simulated engine busy time for static load balancing
            t_eng = {"A": 0.0, "V": 0.0}

            def assign_duo():
                """Pick engines for the two jobs of a half-duo."""
                d = t_eng["V"] - t_eng["A"]
                if d > ACT_RELU_1024:
                    return "A", "A"
                if d < -DVE_RELU_1024:
                    return "V", "V"
                return "A", "V"

            def ps_tile(eng):
                pool = papool if eng == "A" else pvpool
                return pool.tile([128, JD], FP32, name=f"ps{eng}",
                                 tag=f"ps{eng}")

            def emit_evac(eng, ps, out_ap, bias_ap):
                if eng == "A":
                    nc.scalar.activation(out_ap, ps[:, :], Relu,
                                         bias=bias_ap, scale=1.0)
                    t_eng["A"] += ACT_RELU_1024
                else:
                    nc.vector.tensor_scalar(out_ap, ps[:, :], bias_ap, 0.0,
                                            ADD, MAX)
                    t_eng["V"] += DVE_RELU_1024

            def l0_fill_mms(ps, p, half, xg_t, ccl):
                g, j0 = p // 2, 2 * (p % 2)
                cc = 2 * half + ccl
                fs = slice(cc * F, (cc + 1) * F)
                os_ = slice(ccl * F, (ccl + 1) * F)
                for (j, colh) in ((j0, 0), (j0 + 1, 64)):
                    nc.tensor.matmul(
                        out=ps[colh:colh + 64, os_],
                        lhsT=w0_t[32 * j:32 * j + 3, 64 * g:64 * (g + 1)],
                        rhs=xg_t[32 * j:32 * j + 3, fs],
                        start=True, stop=True,
                        tile_position=(32 * j, colh))

            def mid_fill_mms(ps, p, half, l, hc, ccl):
                col = (l * 8 + p) * 64
                cc = 2 * half + ccl
                fs = slice(cc * F, (cc + 1) * F)
                os_ = slice(ccl * F, (ccl + 1) * F)
                if p % 2 == 0:
                    nc.tensor.matmul(out=ps[0:64, os_],
                                     lhsT=wm_t[0:64, col:col + 64],
                                     rhs=hc[p][0:64, fs],
                                     start=True, stop=True)
                    nc.tensor.matmul(out=ps[64:128, os_],
                                     lhsT=wm_t[64:128, col:col + 64],
                                     rhs=hc[p][64:128, fs],
                                     start=True, stop=True)
                else:
                    nc.tensor.matmul(out=ps[64:128, os_],
                                     lhsT=wm_t[0:64, col:col + 64],
                                     rhs=hc[p][0:64, fs],
                                     start=True, stop=True)
                    nc.tensor.matmul(out=ps[0:64, os_],
                                     lhsT=wm_t[64:128, col:col + 64],
                                     rhs=hc[p][64:128, fs],
                                     start=True, stop=True)

            def emit_half_duo(fill_fn0, fill_fn1, out0, out1, bias0, bias1):
                """Two [128,JD] jobs (even pair, odd pair) of one half-duo."""
                e0, e1 = assign_duo()
                ps0 = ps_tile(e0)
                ps1 = ps_tile(e1)
                # interleave the two jobs' fill MMs chunk-by-chunk so the
                # even/odd quadrant placements overlap in the PE array
                for ccl in range(2):
                    fill_fn0(ps0, ccl)
                    fill_fn1(ps1, ccl)
                emit_evac(e0, ps0, out0, bias0)
                emit_evac(e1, ps1, out1, bias1)

            def emit_matvec(h6, msg):
                mv = pvpool.tile([128, JD], FP32, name="mv", tag="psV")
                for p in range(8):
                    for cc in range(4):
                        nc.tensor.matmul(
                            out=mv[32 * cc:32 * cc + 32, 0:F],
                            lhsT=vt_t[:, 32 * p:32 * p + 32],
                            rhs=h6[p][:, cc * F:(cc + 1) * F],
                            start=(p == 0), stop=(p == 7),
                            tile_position=(0, 32 * cc),
                            skip_group_check=True)
                out_sb = opool.tile([128, F], FP32, name="osb", tag="osb")
                nc.vector.tensor_copy(out_sb[0:97, :], mv[0:97, 0:F])
                t_eng["V"] += MV_COPY_512
                ou_v = out_sb.rearrange("(a b) f -> a b f", b=32)[:, 0:1, :]
                nc.sync.dma_start(out=out_h[msg, :, :], in_=ou_v)

            pending_mv = None  # (h6 tiles, sg index)
            for sg in range(NSG):
                s0 = sg * SG
                glocal = sg // 2

                xg = []
                for g in range(4):
                    xt = xpool.tile([128, SG], FP16, name=f"x{g}", tag=f"x{g}")
                    for j in range(4):
                        nc.sync.dma_start(
                            out=xt[32 * j:32 * j + 3, :],
                            in_=x3_h[g, 3 * j:3 * j + 3, s0:s0 + SG])
                    xg.append(xt)
                    if sg == 0 and g == 0:
                        nc.sync.dma_start(out=bm_t[:, :], in_=bm_h[:, :])
                        nc.sync.dma_start(out=wm_t[:, :], in_=wm_h[:, :])
                        nc.sync.dma_start(out=vt_t[:, :], in_=vt_h[:, :])

                # ---- layer 0 ----
                h_cur = [hpool.tile([128, SG], FP16, name=f"h{p}_a", tag=f"h{p}_a")
                         for p in range(8)]
                for q in range(4):
                    p0, p1 = 2 * q, 2 * q + 1
                    for half in range(2):
                        hs = slice(half * JD, (half + 1) * JD)
                        emit_half_duo(
                            lambda ps, ccl, p=p0: l0_fill_mms(ps, p, half, xg[q], ccl),
                            lambda ps, ccl, p=p1: l0_fill_mms(ps, p, half, xg[q], ccl),
                            h_cur[p0][:, hs], h_cur[p1][:, hs],
                            b0e_t[:, p0 * 2 + glocal:p0 * 2 + glocal + 1],
                            b0e_t[:, p1 * 2 + glocal:p1 * 2 + glocal + 1])
                    if q == 0 and pending_mv is not None:
                        emit_matvec(*pending_mv)
                        pending_mv = None

                # ---- mid layers l=0..5 ----
                for l in range(6):
                    suf = "b" if l % 2 == 0 else "a"
                    h_nxt = [hpool.tile([128, SG], FP16, name=f"h{p}_{suf}",
                                        tag=f"h{p}_{suf}") for p in range(8)]
                    for q in range(4):
                        p0, p1 = 2 * q, 2 * q + 1
                        for half in range(2):
                            hs = slice(half * JD, (half + 1) * JD)
                            emit_half_duo(
                                lambda ps, ccl, p=p0: mid_fill_mms(ps, p, half, l, h_cur, ccl),
                                lambda ps, ccl, p=p1: mid_fill_mms(ps, p, half, l, h_cur, ccl),
                                h_nxt[p0][:, hs], h_nxt[p1][:, hs],
                                bm_t[:, l * 8 + p0:l * 8 + p0 + 1],
                                bm_t[:, l * 8 + p1:l * 8 + p1 + 1])
                    h_cur = h_nxt

                pending_mv = (h_cur, sg)
            emit_matvec(*pending_mv)
    nc.finalize()
    return nc


def kernel(xyz, joints, W0, b0, Wmid, bmid, Wf, bf):
    in_maps, c = _host_prep(xyz, joints, W0, b0, Wmid, bmid, Wf, bf)
    key = "nc"
    if key not in _CACHE:
        _CACHE[key] = _build()
    nc = _CACHE[key]
    res = run_bass_kernel_spmd(nc, in_maps, core_ids=list(range(NCORES)))
    s = np.concatenate([r["out"].reshape(-1) for r in res.results])
    return np.tanh(s + c).reshape(N, 1).astype(np.float32)
